# revision 1
# baseline (speedup 1.0000x reference)
"""Multi-head attention (batched, key-padding mask) Trainium2 Bass kernel.

Problem: nn_MultiHeadBatched
  q,k,v: [B=4, S=2048, E=1024] fp32; mask: [B, 2048] int32 (key padding)
  16 heads, head_dim 64; torch-Linear style q/k/v/out projections.

Sharding (8 cores): core c handles batch b=c//2 and head group hg=c%2
(8 heads each).  q/k/v projections are column-parallel over the head
group; out-projection is row-parallel — each core produces a partial
[E, Sq] output and the host sums the two partials per batch (+ bo).

Key structure (single NeuronCore program, SPMD over 8 cores):
  - Host compacts the KV sequence per batch to the valid (mask!=0)
    positions and pads to a multiple of 128 (SKV).  Padded positions get
    an additive -1e30 bias before exp (folded into the ScalarE exp
    activation), contributing exactly 0 — this skips ~45% of the
    attention work for Bernoulli(0.5) masks.
  - All matmuls in bf16 with fp32 PSUM accumulation.
  - Scores are computed transposed ([kv, q]); the softmax normalizer Z
    comes from an all-ones column appended to each head's V (row 64 of
    the AV accumulation), avoiding any cross-partition reduction.
  - No max-subtraction in softmax: scores/8 are ~N(0,1), far from fp32
    overflow, and max-subtraction is mathematically a no-op here.
  - Head-slot software pipeline: slot h runs QK^T+exp for head h while
    the PE also runs AV for head h-1, interleaved kv-chunk by kv-chunk
    so the PE never idles long enough for the HAM clock gate to
    re-throttle.  The V projection fills slot 0 (which has no AV yet).
"""

import os
import sys

import numpy as np

sys.path.insert(0, "/opt/trn_rl_repo")

import concourse.bass as bass
import concourse.bacc as bacc
import concourse.mybir as mybir
import concourse.tile as tile
from concourse import bass_utils

import ml_dtypes

BF16 = ml_dtypes.bfloat16

B, SQ, E = 4, 2048, 1024
H_TOT, D = 16, 64
HPC = H_TOT // 2            # heads per core (head-group split in 2)
DHC = HPC * D               # 512 projected channels per core
NE = E // 128               # contraction chunks
NDH = DHC // 128            # dh chunks per core
NTS = SQ // 512             # 512-wide q strips
NEG = -1.0e30
SCALE = D ** -0.5

N_CORES = 8

_PROGRAM_CACHE = {}
LAST_RESULTS = None


def _chunks512(n):
    out = []
    o = 0
    while o < n:
        w = min(512, n - o)
        out.append((o, w))
        o += w
    return out


def build_program(skv):
    """Build + compile the single-core SPMD Bass program for padded KV
    length `skv` (multiple of 128)."""
    if skv in _PROGRAM_CACHE:
        return _PROGRAM_CACHE[skv]

    nkv = skv // 128
    dt = mybir.dt

    nc = bacc.Bacc(
        "TRN2",
        target_bir_lowering=False,
        debug=False,
        enable_asserts=False,
        num_devices=N_CORES,
    )

    # DRAM I/O (per-core shapes)
    qT = nc.dram_tensor("qT", [E, SQ], dt.bfloat16, kind="ExternalInput").ap()
    kT = nc.dram_tensor("kT", [E, skv], dt.bfloat16, kind="ExternalInput").ap()
    vT = nc.dram_tensor("vT", [E, skv], dt.bfloat16, kind="ExternalInput").ap()
    wqT = nc.dram_tensor("wqT", [E, DHC], dt.bfloat16, kind="ExternalInput").ap()
    wkT = nc.dram_tensor("wkT", [E, DHC], dt.bfloat16, kind="ExternalInput").ap()
    wvT = nc.dram_tensor("wvT", [E, DHC], dt.bfloat16, kind="ExternalInput").ap()
    woT = nc.dram_tensor("woT", [DHC, E], dt.bfloat16, kind="ExternalInput").ap()
    mb = nc.dram_tensor("mb", [128, nkv], dt.float32, kind="ExternalInput").ap()
    outT = nc.dram_tensor("outT", [E, SQ], dt.float32, kind="ExternalOutput").ap()

    ts = bass.ts
    kvchunks = _chunks512(skv)

    with tile.TileContext(nc) as tc:
        with tc.tile_pool(name="persist", bufs=1) as pp:
            # Persistent SBUF tensors
            wq_sb = [pp.tile([128, DHC], dt.bfloat16, name=f"wq{e}", tag=f"wq{e}") for e in range(NE)]
            wk_sb = [pp.tile([128, DHC], dt.bfloat16, name=f"wk{e}", tag=f"wk{e}") for e in range(NE)]
            wv_sb = [pp.tile([128, DHC], dt.bfloat16, name=f"wv{e}", tag=f"wv{e}") for e in range(NE)]
            wo_sb = [pp.tile([128, E], dt.bfloat16, name=f"wo{c}", tag=f"wo{c}") for c in range(NDH)]
            qh_sb = [pp.tile([128, SQ], dt.bfloat16, name=f"qh{c}", tag=f"qh{c}") for c in range(NDH)]
            kh_sb = [pp.tile([128, skv], dt.bfloat16, name=f"kh{c}", tag=f"kh{c}") for c in range(NDH)]
            # V with per-head interleaved ones column: [kv, 8*(64+1)]
            va_sb = [pp.tile([128, HPC * (D + 1)], dt.bfloat16, name=f"va{j}", tag=f"va{j}") for j in range(nkv)]
            aall_sb = [pp.tile([128, SQ], dt.bfloat16, name=f"aall{c}", tag=f"aall{c}") for c in range(NDH)]
            mb_sb = pp.tile([128, nkv], dt.float32, name="mbt", tag="mbt")

            # ones columns of the augmented V (bf16 1.0)
            for j in range(nkv):
                nc.gpsimd.memset(va_sb[j][:, D::D + 1], 1.0)

            # v inputs live until the end of attention slot 0
            vip = tc.alloc_tile_pool(name="vinp", bufs=1)
            v_sb = [vip.tile([128, skv], dt.bfloat16, name=f"v{e}", tag=f"v{e}") for e in range(NE)]

            # ---------------- Q/K projections ----------------
            with (
                tc.tile_pool(name="inp", bufs=1) as ip,
                tc.tile_pool(name="qpp", bufs=4, space="PSUM") as qpp,
                tc.tile_pool(name="kpp", bufs=2, space="PSUM") as kpp,
            ):
                q_sb = [ip.tile([128, SQ], dt.bfloat16, name=f"q{e}", tag=f"q{e}") for e in range(NE)]
                k_sb = [ip.tile([128, skv], dt.bfloat16, name=f"k{e}", tag=f"k{e}") for e in range(NE)]
                # DMA order matches first-use order
                for e in range(NE):
                    nc.sync.dma_start(wq_sb[e][:], wqT[ts(e, 128), :])
                    nc.sync.dma_start(q_sb[e][:], qT[ts(e, 128), :])
                for e in range(NE):
                    nc.sync.dma_start(wk_sb[e][:], wkT[ts(e, 128), :])
                    nc.sync.dma_start(k_sb[e][:], kT[ts(e, 128), :])
                for e in range(NE):
                    nc.sync.dma_start(wv_sb[e][:], wvT[ts(e, 128), :])
                    nc.sync.dma_start(v_sb[e][:], vT[ts(e, 128), :])
                nc.sync.dma_start(mb_sb[:], mb[:])
                for c in range(NDH):
                    nc.sync.dma_start(wo_sb[c][:], woT[ts(c, 128), :])

                for c in range(NDH):
                    # Q projection: QhT[dh, t] (transposed heads)
                    qps = [qpp.tile([128, 512], dt.float32, name=f"qps{t}", tag="qps") for t in range(NTS)]
                    for e in range(NE):
                        for t in range(NTS):
                            nc.tensor.matmul(
                                qps[t][:], wq_sb[e][:, ts(c, 128)], q_sb[e][:, ts(t, 512)],
                                start=(e == 0), stop=(e == NE - 1),
                            )
                    for t in range(NTS):
                        nc.vector.tensor_copy(qh_sb[c][:, ts(t, 512)], qps[t][:])

                    # K projection for the same dh chunk
                    for (o, w) in kvchunks:
                        kps = kpp.tile([128, 512], dt.float32, name="kps", tag="kps")
                        for e in range(NE):
                            nc.tensor.matmul(
                                kps[:, 0:w], wk_sb[e][:, ts(c, 128)], k_sb[e][:, o:o + w],
                                start=(e == 0), stop=(e == NE - 1),
                            )
                        nc.vector.tensor_copy(kh_sb[c][:, o:o + w], kps[:, 0:w])

            # ---------------- attention ----------------
            with (
                tc.tile_pool(name="ppool", bufs=2) as ppool,
                tc.tile_pool(name="npool", bufs=2) as npool,
                tc.tile_pool(name="scp", bufs=1, space="PSUM") as scp,
            ):
                vpp = tc.alloc_tile_pool(name="vpp", bufs=2, space="PSUM")
                app = None
                p_prev = None
                for h in range(HPC + 1):
                    if h < HPC:
                        c, r = h // 2, h % 2
                        qh_h = qh_sb[c][r * 64:(r + 1) * 64, :]
                        kh_h = kh_sb[c][r * 64:(r + 1) * 64, :]
                        p_cur = []
                    if h > 0:
                        hp = h - 1
                        cp, rp = hp // 2, hp % 2
                        # pass p covers q strips (2p, 2p+1); Z in row 64
                        a2 = [app.tile([D + 1, 1024], dt.float32, name=f"a2_{p}", tag="aps")
                              for p in range(2)]

                    for j in range(nkv):
                        # Dependency-free filler LDWEIGHTS keep the PE array
                        # active through short ACT-bound waits (HAM stays 8/8).
                        for _ in range(2):
                            nc.tensor.ldweights(weights=wq_sb[0][:, 0:128])
                        if h < HPC:
                            # scores^T -> exp -> P[j]  [kv, q]
                            pt = ppool.tile([128, SQ], dt.bfloat16, name=f"p{j}", tag=f"p{j}")
                            p_cur.append(pt)
                            sc = scp.tile([128, SQ], dt.float32, name="sc", tag="sc")
                            for s in range(NTS):
                                nc.tensor.matmul(
                                    sc[:, ts(s, 512)],
                                    kh_h[:, ts(j, 128)],
                                    qh_h[:, ts(s, 512)],
                                    start=True, stop=True,
                                )
                            nc.scalar.activation(
                                pt[:], sc[:],
                                mybir.ActivationFunctionType.Exp,
                                bias=mb_sb[:, j:j + 1], scale=SCALE,
                            )
                        if h == 0:
                            # V projection chunk j fills slot 0's PE slack:
                            # Vh[kv, dh], written head-interleaved into va_sb
                            vps = vpp.tile([128, DHC], dt.float32, name="vps", tag="vps")
                            for e in range(NE):
                                nc.tensor.matmul(
                                    vps[:], v_sb[e][:, ts(j, 128)], wv_sb[e][:],
                                    start=(e == 0), stop=(e == NE - 1),
                                )
                            dst = va_sb[j].rearrange("p (h x) -> p h x", x=D + 1)[:, :, 0:D]
                            src = vps.rearrange("p (h x) -> p h x", x=D)
                            nc.vector.tensor_copy(dst, src)
                        if h > 0:
                            # AV for previous head, j-major accumulation
                            for p in range(2):
                                for si in range(2):
                                    nc.tensor.matmul(
                                        a2[p][:, ts(si, 512)],
                                        va_sb[j][:, hp * (D + 1):(hp + 1) * (D + 1)],
                                        p_prev[j][:, ts(2 * p + si, 512)],
                                        start=(j == 0), stop=(j == nkv - 1),
                                    )

                    if h == 0:
                        vpp.release()
                        app = tc.alloc_tile_pool(name="app", bufs=2, space="PSUM")
                    if h > 0:
                        for p in range(2):
                            # copy A (+Z row) out of PSUM right away so the
                            # banks free for the next head's AV; the slow
                            # normalization then runs entirely from SBUF.
                            au = npool.tile([D + 1, 1024], dt.float32, name="au", tag="au", bufs=4)
                            nc.vector.tensor_copy(au[:], a2[p][:])
                            for si in range(2):
                                s = 2 * p + si
                                rz = npool.tile([1, 512], dt.float32, name="rz", tag="rz", bufs=4)
                                nc.vector.reciprocal(rz[:], au[D:D + 1, ts(si, 512)])
                                rb = npool.tile([64, 512], dt.float32, name="rb", tag="rb", bufs=4)
                                nc.gpsimd.partition_broadcast(rb[:], rz[:])
                                nc.gpsimd.tensor_mul(
                                    aall_sb[cp][rp * 64:(rp + 1) * 64, ts(s, 512)],
                                    au[0:D, ts(si, 512)], rb[:],
                                )
                    p_prev = p_cur if h < HPC else None
                app.release()

            # ---------------- out projection ----------------
            with (
                tc.tile_pool(name="opool", bufs=4) as opool,
                tc.tile_pool(name="opp", bufs=4, space="PSUM") as opp,
            ):
                for eo in range(NE):
                    for t in range(NTS):
                        ops = opp.tile([128, 512], dt.float32, name="ops", tag="ops")
                        for c in range(NDH):
                            nc.tensor.matmul(
                                ops[:], wo_sb[c][:, ts(eo, 128)], aall_sb[c][:, ts(t, 512)],
                                start=(c == 0), stop=(c == NDH - 1),
                            )
                        ob = opool.tile([128, 512], dt.float32, name="ob", tag="ob")
                        nc.vector.tensor_copy(ob[:], ops[:])
                        nc.sync.dma_start(outT[ts(eo, 128), ts(t, 512)], ob[:])
            vip.release()

    nc.compile()
    _PROGRAM_CACHE[skv] = nc
    return nc


def make_in_maps(q, k, v, mask, Wq, Wk, Wv, Wo, skv):
    """Host-side shard/compact/transpose/cast. Returns per-core input dicts."""
    in_maps = []
    valid = mask != 0
    for core in range(N_CORES):
        b, hg = core // 2, core % 2
        idx = np.nonzero(valid[b])[0]
        cnt = len(idx)

        kc = np.zeros((skv, E), np.float32)
        vc = np.zeros((skv, E), np.float32)
        kc[:cnt] = k[b][idx]
        vc[:cnt] = v[b][idx]

        mbias = np.zeros((skv,), np.float32)
        mbias[cnt:] = NEG
        # [128, nkv]: column j = kv chunk j
        mb2 = np.ascontiguousarray(mbias.reshape(-1, 128).T)

        rows = slice(hg * DHC, (hg + 1) * DHC)
        in_maps.append(dict(
            qT=np.ascontiguousarray(q[b].T).astype(BF16),
            kT=np.ascontiguousarray(kc.T).astype(BF16),
            vT=np.ascontiguousarray(vc.T).astype(BF16),
            wqT=np.ascontiguousarray(Wq[rows, :].T).astype(BF16),
            wkT=np.ascontiguousarray(Wk[rows, :].T).astype(BF16),
            wvT=np.ascontiguousarray(Wv[rows, :].T).astype(BF16),
            woT=np.ascontiguousarray(Wo[:, rows].T).astype(BF16),
            mb=mb2,
        ))
    return in_maps


def _numpy_fallback(q, k, v, mask, Wq, bq, Wk, bk, Wv, bv, Wo, bo):
    out = np.zeros((B, SQ, E), np.float32)
    for b in range(B):
        qh = (q[b] @ Wq.T + bq).reshape(SQ, H_TOT, D).transpose(1, 0, 2)
        kh = (k[b] @ Wk.T + bk).reshape(-1, H_TOT, D).transpose(1, 0, 2)
        vh = (v[b] @ Wv.T + bv).reshape(-1, H_TOT, D).transpose(1, 0, 2)
        att = np.einsum("hqd,hkd->hqk", qh, kh) * SCALE
        valid = mask[b] != 0
        if not valid.any():
            out[b] = bo
            continue
        att = np.where(valid[None, None, :], att, -np.inf)
        att = att - att.max(-1, keepdims=True)
        att = np.exp(att)
        att /= att.sum(-1, keepdims=True)
        o = np.einsum("hqk,hkd->hqd", att, vh)
        o = o.transpose(1, 0, 2).reshape(SQ, E)
        out[b] = o @ Wo.T + bo
    return out


def kernel(**inputs):
    global LAST_RESULTS
    q = np.asarray(inputs["q"], np.float32)
    k = np.asarray(inputs["k"], np.float32)
    v = np.asarray(inputs["v"], np.float32)
    mask = np.asarray(inputs["mask"])
    Wq, bq = np.asarray(inputs["Wq"], np.float32), np.asarray(inputs["bq"], np.float32)
    Wk, bk = np.asarray(inputs["Wk"], np.float32), np.asarray(inputs["bk"], np.float32)
    Wv, bv = np.asarray(inputs["Wv"], np.float32), np.asarray(inputs["bv"], np.float32)
    Wo, bo = np.asarray(inputs["Wo"], np.float32), np.asarray(inputs["bo"], np.float32)

    if any(np.abs(x).max() > 0 for x in (bq, bk, bv)):
        # q/k/v biases are zero in this problem's setup; a nonzero bias
        # would need the augmented-contraction path, so fall back.
        return _numpy_fallback(q, k, v, mask, Wq, bq, Wk, bk, Wv, bv, Wo, bo)

    valid = mask != 0
    counts = valid.sum(axis=1)
    if counts.max() == 0:
        return np.broadcast_to(bo, (B, SQ, E)).astype(np.float32).copy()

    skv = int(-(-counts.max() // 128) * 128)
    nc = build_program(skv)
    in_maps = make_in_maps(q, k, v, mask, Wq, Wk, Wv, Wo, skv)

    res = bass_utils.run_bass_kernel_spmd(nc, in_maps, core_ids=list(range(N_CORES)))
    LAST_RESULTS = res

    out = np.empty((B, SQ, E), np.float32)
    for b in range(B):
        if counts[b] == 0:
            out[b] = bo
        else:
            p0 = res.results[2 * b]["outT"]
            p1 = res.results[2 * b + 1]["outT"]
            out[b] = p0.T + p1.T + bo
    return out



# revision 9
# speedup vs baseline: 1.4017x; 1.4017x over previous
"""Multi-head attention (batched, key-padding mask) Trainium2 Bass kernel.

Problem: nn_MultiHeadBatched
  q,k,v: [B=4, S=2048, E=1024] fp32; mask: [B, 2048] int32 (key padding)
  16 heads, head_dim 64; torch-Linear style q/k/v/out projections.

Sharding (8 cores): core c handles batch b=c//2 and head group hg=c%2
(8 heads each).  q/k/v projections are column-parallel over the head
group; out-projection is row-parallel — each core produces a partial
[E, Sq] output and the host sums the two partials per batch (+ bo).

Key structure (single NeuronCore program, SPMD over 8 cores):
  - Host compacts the KV sequence per batch to the valid (mask!=0)
    positions and pads to a multiple of 128 (SKV).  Padded positions get
    an additive -1e30 bias before exp (folded into the ScalarE exp
    activation), contributing exactly 0 — this skips ~45% of the
    attention work for Bernoulli(0.5) masks.
  - All matmuls in bf16 with fp32 PSUM accumulation.
  - Scores are computed transposed ([kv, q]); the softmax normalizer Z
    comes from an all-ones column appended to each head's V (row 64 of
    the AV accumulation), avoiding any cross-partition reduction.
  - No max-subtraction in softmax: scores/8 are ~N(0,1), far from fp32
    overflow, and max-subtraction is mathematically a no-op here.
  - Head-slot software pipeline: slot h runs QK^T+exp for head h while
    the PE also runs AV for head h-1, interleaved kv-chunk by kv-chunk
    so the PE never idles long enough for the HAM clock gate to
    re-throttle.  The V projection fills slot 0 (which has no AV yet).
"""

import os
import sys

import numpy as np

sys.path.insert(0, "/opt/trn_rl_repo")

import concourse.bass as bass
import concourse.bacc as bacc
import concourse.mybir as mybir
import concourse.tile as tile
from concourse import bass_utils

import ml_dtypes

BF16 = ml_dtypes.bfloat16

B, SQ, E = 4, 2048, 1024
H_TOT, D = 16, 64
HPC = H_TOT // 2            # heads per core (head-group split in 2)
DHC = HPC * D               # 512 projected channels per core
NE = E // 128               # contraction chunks
NDH = DHC // 128            # dh chunks per core
NTS = SQ // 512             # 512-wide q strips
NEG = -1.0e30
SCALE = D ** -0.5

N_CORES = 8

_PROGRAM_CACHE = {}
LAST_RESULTS = None


def _chunks512(n):
    out = []
    o = 0
    while o < n:
        w = min(512, n - o)
        out.append((o, w))
        o += w
    return out


def build_program(skv):
    """Build + compile the single-core SPMD Bass program for padded KV
    length `skv` (multiple of 128)."""
    if skv in _PROGRAM_CACHE:
        return _PROGRAM_CACHE[skv]

    nkv = skv // 128
    dt = mybir.dt

    nc = bacc.Bacc(
        "TRN2",
        target_bir_lowering=False,
        debug=False,
        enable_asserts=False,
        num_devices=N_CORES,
    )

    # DRAM I/O (per-core shapes)
    qT = nc.dram_tensor("qT", [E, SQ], dt.bfloat16, kind="ExternalInput").ap()
    kT = nc.dram_tensor("kT", [E, skv], dt.bfloat16, kind="ExternalInput").ap()
    vT = nc.dram_tensor("vT", [E, skv], dt.bfloat16, kind="ExternalInput").ap()
    wqT = nc.dram_tensor("wqT", [E, DHC], dt.bfloat16, kind="ExternalInput").ap()
    wkT = nc.dram_tensor("wkT", [E, DHC], dt.bfloat16, kind="ExternalInput").ap()
    wvT = nc.dram_tensor("wvT", [E, DHC], dt.bfloat16, kind="ExternalInput").ap()
    woT = nc.dram_tensor("woT", [DHC, E], dt.bfloat16, kind="ExternalInput").ap()
    mb = nc.dram_tensor("mb", [128, nkv], dt.float32, kind="ExternalInput").ap()
    outT = nc.dram_tensor("outT", [E, SQ], dt.float32, kind="ExternalOutput").ap()

    ts = bass.ts
    kvchunks = _chunks512(skv)

    with tile.TileContext(nc) as tc:
        with tc.tile_pool(name="persist", bufs=1) as pp:
            # Persistent SBUF tensors
            wq_sb = [pp.tile([128, DHC], dt.bfloat16, name=f"wq{e}", tag=f"wq{e}") for e in range(NE)]
            wk_sb = [pp.tile([128, DHC], dt.bfloat16, name=f"wk{e}", tag=f"wk{e}") for e in range(NE)]
            wv_sb = [pp.tile([128, DHC], dt.bfloat16, name=f"wv{e}", tag=f"wv{e}") for e in range(NE)]
            wo_sb = [pp.tile([128, E], dt.bfloat16, name=f"wo{c}", tag=f"wo{c}") for c in range(NDH)]
            qh_sb = [pp.tile([128, SQ], dt.bfloat16, name=f"qh{c}", tag=f"qh{c}") for c in range(NDH)]
            kh_sb = [pp.tile([128, skv], dt.bfloat16, name=f"kh{c}", tag=f"kh{c}") for c in range(NDH)]
            # V augmented per head to [kv, 64 V | 64 ones]: the 64 ones
            # columns replicate the softmax denominator Z into PSUM
            # partitions 64-127, partition-aligned with the A rows, so the
            # normalization is one recip + one elementwise mul per strip
            # (no cross-partition broadcast op).
            va_sb = [pp.tile([128, HPC * 2 * D], dt.bfloat16, name=f"va{j}", tag=f"va{j}") for j in range(nkv)]
            aall_sb = [pp.tile([128, SQ], dt.bfloat16, name=f"aall{c}", tag=f"aall{c}") for c in range(NDH)]
            mb_sb = pp.tile([128, nkv], dt.float32, name="mbt", tag="mbt")

            # ones half-blocks of the augmented V (bf16 1.0)
            for j in range(nkv):
                va3 = va_sb[j].rearrange("p (h x) -> p h x", x=2 * D)
                nc.gpsimd.memset(va3[:, :, D:2 * D], 1.0)

            # v inputs live until the end of attention slot 0
            vip = tc.alloc_tile_pool(name="vinp", bufs=1)
            v_sb = [vip.tile([128, skv], dt.bfloat16, name=f"v{e}", tag=f"v{e}") for e in range(NE)]

            # ---------------- Q/K projections ----------------
            with (
                tc.tile_pool(name="inp", bufs=1) as ip,
                tc.tile_pool(name="qpp", bufs=4, space="PSUM") as qpp,
                tc.tile_pool(name="kpp", bufs=2, space="PSUM") as kpp,
            ):
                q_sb = [ip.tile([128, SQ], dt.bfloat16, name=f"q{e}", tag=f"q{e}") for e in range(NE)]
                k_sb = [ip.tile([128, skv], dt.bfloat16, name=f"k{e}", tag=f"k{e}") for e in range(NE)]
                # DMA order matches first-use order
                for e in range(NE):
                    nc.sync.dma_start(wq_sb[e][:], wqT[ts(e, 128), :])
                    nc.sync.dma_start(q_sb[e][:], qT[ts(e, 128), :])
                for e in range(NE):
                    nc.sync.dma_start(wk_sb[e][:], wkT[ts(e, 128), :])
                    nc.sync.dma_start(k_sb[e][:], kT[ts(e, 128), :])
                for e in range(NE):
                    nc.sync.dma_start(wv_sb[e][:], wvT[ts(e, 128), :])
                    nc.sync.dma_start(v_sb[e][:], vT[ts(e, 128), :])
                nc.sync.dma_start(mb_sb[:], mb[:])
                for c in range(NDH):
                    nc.sync.dma_start(wo_sb[c][:], woT[ts(c, 128), :])

                for c in range(NDH):
                    # Q projection: QhT[dh, t] (transposed heads)
                    qps = [qpp.tile([128, 512], dt.float32, name=f"qps{t}", tag="qps") for t in range(NTS)]
                    for e in range(NE):
                        for t in range(NTS):
                            nc.tensor.matmul(
                                qps[t][:], wq_sb[e][:, ts(c, 128)], q_sb[e][:, ts(t, 512)],
                                start=(e == 0), stop=(e == NE - 1),
                            )
                    for t in range(NTS):
                        nc.vector.tensor_copy(qh_sb[c][:, ts(t, 512)], qps[t][:])

                    # K projection for the same dh chunk
                    for (o, w) in kvchunks:
                        kps = kpp.tile([128, 512], dt.float32, name="kps", tag="kps")
                        for e in range(NE):
                            nc.tensor.matmul(
                                kps[:, 0:w], wk_sb[e][:, ts(c, 128)], k_sb[e][:, o:o + w],
                                start=(e == 0), stop=(e == NE - 1),
                            )
                        nc.vector.tensor_copy(kh_sb[c][:, o:o + w], kps[:, 0:w])

            # ---------------- attention ----------------
            with (
                tc.tile_pool(name="ppool", bufs=2) as ppool,
                tc.tile_pool(name="npool", bufs=2) as npool,
                tc.tile_pool(name="scp", bufs=1, space="PSUM") as scp,
            ):
                vpp = tc.alloc_tile_pool(name="vpp", bufs=2, space="PSUM")
                app = None
                p_prev = None
                for h in range(HPC + 1):
                    if h < HPC:
                        c, r = h // 2, h % 2
                        qh_h = qh_sb[c][r * 64:(r + 1) * 64, :]
                        kh_h = kh_sb[c][r * 64:(r + 1) * 64, :]
                        p_cur = []
                    if h > 0:
                        hp = h - 1
                        cp, rp = hp // 2, hp % 2
                        # pass p covers q strips (2p, 2p+1); Z replicated in
                        # rows 64-127
                        a2 = [app.tile([128, 1024], dt.float32, name=f"a2_{p}", tag="aps")
                              for p in range(2)]

                    for j in range(nkv):
                        # Dependency-free filler LDWEIGHTS keep the PE array
                        # active through short ACT-bound waits (HAM stays 8/8).
                        for _ in range(2):
                            nc.tensor.ldweights(weights=wq_sb[0][:, 0:128])
                        if h < HPC:
                            # scores^T -> exp -> P[j]  [kv, q]
                            pt = ppool.tile([128, SQ], dt.bfloat16, name=f"p{j}", tag=f"p{j}")
                            p_cur.append(pt)
                            sc = scp.tile([128, SQ], dt.float32, name="sc", tag="sc")
                            for s in range(NTS):
                                nc.tensor.matmul(
                                    sc[:, ts(s, 512)],
                                    kh_h[:, ts(j, 128)],
                                    qh_h[:, ts(s, 512)],
                                    start=True, stop=True,
                                )
                            nc.scalar.activation(
                                pt[:], sc[:],
                                mybir.ActivationFunctionType.Exp,
                                bias=mb_sb[:, j:j + 1], scale=SCALE,
                            )
                        if h == 0:
                            # V projection chunk j fills slot 0's PE slack:
                            # Vh[kv, dh], written head-interleaved into va_sb
                            vps = vpp.tile([128, DHC], dt.float32, name="vps", tag="vps")
                            for e in range(NE):
                                nc.tensor.matmul(
                                    vps[:], v_sb[e][:, ts(j, 128)], wv_sb[e][:],
                                    start=(e == 0), stop=(e == NE - 1),
                                )
                            dst = va_sb[j].rearrange("p (h x) -> p h x", x=2 * D)[:, :, 0:D]
                            src = vps.rearrange("p (h x) -> p h x", x=D)
                            nc.vector.tensor_copy(dst, src)
                        if h > 0:
                            # AV for previous head, j-major accumulation
                            for p in range(2):
                                for si in range(2):
                                    nc.tensor.matmul(
                                        a2[p][:, ts(si, 512)],
                                        va_sb[j][:, hp * 2 * D:(hp + 1) * 2 * D],
                                        p_prev[j][:, ts(2 * p + si, 512)],
                                        start=(j == 0), stop=(j == nkv - 1),
                                    )

                    if h == 0:
                        vpp.release()
                        app = tc.alloc_tile_pool(name="app", bufs=2, space="PSUM")
                    if h > 0:
                        for p in range(2):
                            # Z replicas (PSUM rows 64-127) -> base-0 SBUF
                            # via a standard cross-partition copy; the
                            # custom-DVE fast reciprocal needs base-0
                            # operands (ISA lowering drops base_partition).
                            # Then one DVE mul reads A straight from PSUM.
                            zt = npool.tile([64, 1024], dt.float32, name="zt", tag="zt", bufs=2)
                            nc.vector.tensor_copy(zt[:], a2[p][D:2 * D, :])
                            rz = npool.tile([64, 1024], dt.float32, name="rz", tag="rz", bufs=2)
                            nc.vector.reciprocal_approx_fast(rz[:], zt[:])
                            nc.vector.tensor_mul(
                                aall_sb[cp][rp * 64:(rp + 1) * 64, ts(p, 1024)],
                                a2[p][0:D, :], rz[:],
                            )
                    p_prev = p_cur if h < HPC else None
                app.release()

            # ---------------- out projection ----------------
            with (
                tc.tile_pool(name="opool", bufs=4) as opool,
                tc.tile_pool(name="opp", bufs=4, space="PSUM") as opp,
            ):
                for eo in range(NE):
                    for t in range(NTS):
                        ops = opp.tile([128, 512], dt.float32, name="ops", tag="ops")
                        for c in range(NDH):
                            nc.tensor.matmul(
                                ops[:], wo_sb[c][:, ts(eo, 128)], aall_sb[c][:, ts(t, 512)],
                                start=(c == 0), stop=(c == NDH - 1),
                            )
                        ob = opool.tile([128, 512], dt.float32, name="ob", tag="ob")
                        nc.vector.tensor_copy(ob[:], ops[:])
                        nc.sync.dma_start(outT[ts(eo, 128), ts(t, 512)], ob[:])
            vip.release()

    nc.compile()
    _PROGRAM_CACHE[skv] = nc
    return nc


def make_in_maps(q, k, v, mask, Wq, Wk, Wv, Wo, skv):
    """Host-side shard/compact/transpose/cast. Returns per-core input dicts."""
    in_maps = []
    valid = mask != 0
    for core in range(N_CORES):
        b, hg = core // 2, core % 2
        idx = np.nonzero(valid[b])[0]
        cnt = len(idx)

        kc = np.zeros((skv, E), np.float32)
        vc = np.zeros((skv, E), np.float32)
        kc[:cnt] = k[b][idx]
        vc[:cnt] = v[b][idx]

        mbias = np.zeros((skv,), np.float32)
        mbias[cnt:] = NEG
        # [128, nkv]: column j = kv chunk j
        mb2 = np.ascontiguousarray(mbias.reshape(-1, 128).T)

        rows = slice(hg * DHC, (hg + 1) * DHC)
        in_maps.append(dict(
            qT=np.ascontiguousarray(q[b].T).astype(BF16),
            kT=np.ascontiguousarray(kc.T).astype(BF16),
            vT=np.ascontiguousarray(vc.T).astype(BF16),
            wqT=np.ascontiguousarray(Wq[rows, :].T).astype(BF16),
            wkT=np.ascontiguousarray(Wk[rows, :].T).astype(BF16),
            wvT=np.ascontiguousarray(Wv[rows, :].T).astype(BF16),
            woT=np.ascontiguousarray(Wo[:, rows].T).astype(BF16),
            mb=mb2,
        ))
    return in_maps


def _numpy_fallback(q, k, v, mask, Wq, bq, Wk, bk, Wv, bv, Wo, bo):
    out = np.zeros((B, SQ, E), np.float32)
    for b in range(B):
        qh = (q[b] @ Wq.T + bq).reshape(SQ, H_TOT, D).transpose(1, 0, 2)
        kh = (k[b] @ Wk.T + bk).reshape(-1, H_TOT, D).transpose(1, 0, 2)
        vh = (v[b] @ Wv.T + bv).reshape(-1, H_TOT, D).transpose(1, 0, 2)
        att = np.einsum("hqd,hkd->hqk", qh, kh) * SCALE
        valid = mask[b] != 0
        if not valid.any():
            out[b] = bo
            continue
        att = np.where(valid[None, None, :], att, -np.inf)
        att = att - att.max(-1, keepdims=True)
        att = np.exp(att)
        att /= att.sum(-1, keepdims=True)
        o = np.einsum("hqk,hkd->hqd", att, vh)
        o = o.transpose(1, 0, 2).reshape(SQ, E)
        out[b] = o @ Wo.T + bo
    return out


def kernel(**inputs):
    global LAST_RESULTS
    q = np.asarray(inputs["q"], np.float32)
    k = np.asarray(inputs["k"], np.float32)
    v = np.asarray(inputs["v"], np.float32)
    mask = np.asarray(inputs["mask"])
    Wq, bq = np.asarray(inputs["Wq"], np.float32), np.asarray(inputs["bq"], np.float32)
    Wk, bk = np.asarray(inputs["Wk"], np.float32), np.asarray(inputs["bk"], np.float32)
    Wv, bv = np.asarray(inputs["Wv"], np.float32), np.asarray(inputs["bv"], np.float32)
    Wo, bo = np.asarray(inputs["Wo"], np.float32), np.asarray(inputs["bo"], np.float32)

    if any(np.abs(x).max() > 0 for x in (bq, bk, bv)):
        # q/k/v biases are zero in this problem's setup; a nonzero bias
        # would need the augmented-contraction path, so fall back.
        return _numpy_fallback(q, k, v, mask, Wq, bq, Wk, bk, Wv, bv, Wo, bo)

    valid = mask != 0
    counts = valid.sum(axis=1)
    if counts.max() == 0:
        return np.broadcast_to(bo, (B, SQ, E)).astype(np.float32).copy()

    skv = int(-(-counts.max() // 128) * 128)
    nc = build_program(skv)
    in_maps = make_in_maps(q, k, v, mask, Wq, Wk, Wv, Wo, skv)

    res = bass_utils.run_bass_kernel_spmd(nc, in_maps, core_ids=list(range(N_CORES)))
    LAST_RESULTS = res

    out = np.empty((B, SQ, E), np.float32)
    for b in range(B):
        if counts[b] == 0:
            out[b] = bo
        else:
            p0 = res.results[2 * b]["outT"]
            p1 = res.results[2 * b + 1]["outT"]
            out[b] = p0.T + p1.T + bo
    return out



# revision 12
# speedup vs baseline: 1.6828x; 1.2005x over previous
"""Multi-head attention (batched, key-padding mask) Trainium2 Bass kernel.

Problem: nn_MultiHeadBatched
  q,k,v: [B=4, S=2048, E=1024] fp32; mask: [B, 2048] int32 (key padding)
  16 heads, head_dim 64; torch-Linear style q/k/v/out projections.

Sharding (8 cores): core c handles batch b=c//2 and head group hg=c%2
(8 heads each).  q/k/v projections are column-parallel over the head
group; out-projection is row-parallel — each core produces a partial
[E, Sq] output and the host sums the two partials per batch (+ bo).

v3 structure (single NeuronCore program, SPMD over 8 cores):
  - Host compacts KV per batch to the valid (mask!=0) positions, padded
    to a multiple of 128 (SKV); pad positions get an additive -1e30 exp
    bias (folded into the ScalarE activation).
  - Scores transposed ([kv, q]); softmax denominator Z from an all-ones
    65th column on each head's V (row 64 of the AV accumulation).
  - Head-slot pipeline with HALF-phases: slot h runs
      AV(h-1, strips 0-1) ; scores+exp(h, q-half 0) ;
      AV(h-1, strips 2-3) ; scores+exp(h, q-half 1)
    so the single-buffered P tiles ([128,2048] per kv chunk) free in
    halves just before exp needs them, and ScalarE stays busy across the
    slot boundary (previous half's exps overlap this slot's AV).
  - Q/K projections for head-pairs 1-3 are background items woven into
    the j-loops, so the exp stream starts ~15us into the kernel.
  - AV is strip-major ([65,512] PSUM, 2 banks) and each strip is
    normalized immediately: PSUM->SBUF copy, reciprocal_approx_fast of
    the Z row, GpSimd partition-broadcast + multiply into aall (bf16).
  - PSUM: scores 2x[128,1024] (4) + AV 2x[65,512] (2) + proj 2x[128,512]
    (2) = 8 banks.
"""

import os
import sys

import numpy as np

sys.path.insert(0, "/opt/trn_rl_repo")

import concourse.bass as bass
import concourse.bacc as bacc
import concourse.mybir as mybir
import concourse.tile as tile
from concourse import bass_utils

import ml_dtypes

BF16 = ml_dtypes.bfloat16

B, SQ, E = 4, 2048, 1024
H_TOT, D = 16, 64
HPC = H_TOT // 2            # heads per core (head-group split in 2)
DHC = HPC * D               # 512 projected channels per core
NE = E // 128               # contraction chunks
NDH = DHC // 128            # dh chunks per core
NTS = SQ // 512             # 512-wide q strips
NEG = -1.0e30
SCALE = D ** -0.5

N_CORES = 8

_PROGRAM_CACHE = {}
LAST_RESULTS = None


def _chunks512(n):
    out = []
    o = 0
    while o < n:
        w = min(512, n - o)
        out.append((o, w))
        o += w
    return out


def build_program(skv):
    """Build + compile the single-core SPMD Bass program for padded KV
    length `skv` (multiple of 128)."""
    if skv in _PROGRAM_CACHE:
        return _PROGRAM_CACHE[skv]

    nkv = skv // 128
    dt = mybir.dt

    nc = bacc.Bacc(
        "TRN2",
        target_bir_lowering=False,
        debug=False,
        enable_asserts=False,
        num_devices=N_CORES,
    )

    # DRAM I/O (per-core shapes)
    qT = nc.dram_tensor("qT", [E, SQ], dt.bfloat16, kind="ExternalInput").ap()
    kT = nc.dram_tensor("kT", [E, skv], dt.bfloat16, kind="ExternalInput").ap()
    vT = nc.dram_tensor("vT", [E, skv], dt.bfloat16, kind="ExternalInput").ap()
    wqT = nc.dram_tensor("wqT", [E, DHC], dt.bfloat16, kind="ExternalInput").ap()
    wkT = nc.dram_tensor("wkT", [E, DHC], dt.bfloat16, kind="ExternalInput").ap()
    wvT = nc.dram_tensor("wvT", [E, DHC], dt.bfloat16, kind="ExternalInput").ap()
    woT = nc.dram_tensor("woT", [DHC, E], dt.bfloat16, kind="ExternalInput").ap()
    mb = nc.dram_tensor("mb", [128, nkv], dt.float32, kind="ExternalInput").ap()
    outT = nc.dram_tensor("outT", [E, SQ], dt.float32, kind="ExternalOutput").ap()

    ts = bass.ts
    kvchunks = _chunks512(skv)

    with tile.TileContext(nc) as tc:
        with tc.tile_pool(name="persist", bufs=1) as pp:
            # Persistent SBUF tensors
            wq_sb = [pp.tile([128, DHC], dt.bfloat16, name=f"wq{e}", tag=f"wq{e}") for e in range(NE)]
            wk_sb = [pp.tile([128, DHC], dt.bfloat16, name=f"wk{e}", tag=f"wk{e}") for e in range(NE)]
            wv_sb = [pp.tile([128, DHC], dt.bfloat16, name=f"wv{e}", tag=f"wv{e}") for e in range(NE)]
            qh_sb = [pp.tile([128, SQ], dt.bfloat16, name=f"qh{c}", tag=f"qh{c}") for c in range(NDH)]
            kh_sb = [pp.tile([128, skv], dt.bfloat16, name=f"kh{c}", tag=f"kh{c}") for c in range(NDH)]
            # V augmented per head to [kv, 64 V | 64 ones]: the ones block
            # replicates the softmax denominator Z into PSUM rows 64-127.
            va_sb = [pp.tile([128, HPC * 2 * D], dt.bfloat16, name=f"va{j}", tag=f"va{j}") for j in range(nkv)]
            aall_sb = [pp.tile([128, SQ], dt.bfloat16, name=f"aall{c}", tag=f"aall{c}") for c in range(NDH)]
            mb_sb = pp.tile([128, nkv], dt.float32, name="mbt", tag="mbt")

            # ones half-blocks of the augmented V (bf16 1.0)
            for j in range(nkv):
                va3 = va_sb[j].rearrange("p (h x) -> p h x", x=2 * D)
                nc.gpsimd.memset(va3[:, :, D:2 * D], 1.0)

            vip = tc.alloc_tile_pool(name="vinp", bufs=1)
            q_sb = [vip.tile([128, SQ], dt.bfloat16, name=f"q{e}", tag=f"q{e}") for e in range(NE)]
            k_sb = [vip.tile([128, skv], dt.bfloat16, name=f"k{e}", tag=f"k{e}") for e in range(NE)]
            v_sb = [vip.tile([128, skv], dt.bfloat16, name=f"v{e}", tag=f"v{e}") for e in range(NE)]

            # DMA order matches first-use order
            for e in range(NE):
                nc.sync.dma_start(wq_sb[e][:], wqT[ts(e, 128), :])
                nc.sync.dma_start(q_sb[e][:], qT[ts(e, 128), :])
            for e in range(NE):
                nc.sync.dma_start(wk_sb[e][:], wkT[ts(e, 128), :])
                nc.sync.dma_start(k_sb[e][:], kT[ts(e, 128), :])
            nc.sync.dma_start(mb_sb[:], mb[:])
            for e in range(NE):
                nc.sync.dma_start(wv_sb[e][:], wvT[ts(e, 128), :])
                nc.sync.dma_start(v_sb[e][:], vT[ts(e, 128), :])

            # PSUM pools, alive for the whole program
            scp = tc.alloc_tile_pool(name="scp", bufs=2, space="PSUM")
            avp = tc.alloc_tile_pool(name="avp", bufs=2, space="PSUM")
            pjp = tc.alloc_tile_pool(name="pjp", bufs=2, space="PSUM")
            npool = tc.alloc_tile_pool(name="npool", bufs=2)

            # ---------------- work items ----------------
            def q_item(c, t):
                qps = pjp.tile([128, 512], dt.float32, name="pj", tag="pj")
                for e in range(NE):
                    nc.tensor.matmul(
                        qps[:], wq_sb[e][:, ts(c, 128)], q_sb[e][:, ts(t, 512)],
                        start=(e == 0), stop=(e == NE - 1),
                    )
                nc.vector.tensor_copy(qh_sb[c][:, ts(t, 512)], qps[:])

            def k_item(c, ci):
                o, w = kvchunks[ci]
                kps = pjp.tile([128, 512], dt.float32, name="pj", tag="pj")
                for e in range(NE):
                    nc.tensor.matmul(
                        kps[:, 0:w], wk_sb[e][:, ts(c, 128)], k_sb[e][:, o:o + w],
                        start=(e == 0), stop=(e == NE - 1),
                    )
                nc.vector.tensor_copy(kh_sb[c][:, o:o + w], kps[:, 0:w])

            def v_item(j):
                vps = pjp.tile([128, 512], dt.float32, name="pj", tag="pj")
                for e in range(NE):
                    nc.tensor.matmul(
                        vps[:], v_sb[e][:, ts(j, 128)], wv_sb[e][:],
                        start=(e == 0), stop=(e == NE - 1),
                    )
                dst = va_sb[j].rearrange("p (h x) -> p h x", x=2 * D)[:, :, 0:D]
                src = vps.rearrange("p (h x) -> p h x", x=D)
                nc.vector.tensor_copy(dst, src)

            # Background queue: Q/K projections for pairs 1-3, popped inside
            # the slot j-loops.  Pair c is fully drained well before slot 2c.
            bg = []
            for c in range(1, NDH):
                for t in range(NTS):
                    bg.append((q_item, c, t))
                for ci in range(len(kvchunks)):
                    bg.append((k_item, c, ci))
            bg.reverse()  # pop() from the end
            nitems = len(bg)
            # cumulative items to drain by end of slot h (pair c by slot 2c-1)
            per_pair = nitems // 3
            bg_deadline = {0: 4, 1: per_pair, 2: per_pair + 4, 3: 2 * per_pair,
                           4: 2 * per_pair + 4, 5: nitems}
            bg_done = 0

            # ---------------- prologue: pair-0 projections ----------------
            for t in range(NTS):
                q_item(0, t)
            for ci in range(len(kvchunks)):
                k_item(0, ci)

            # ---------------- head-slot pipeline ----------------
            def av_strip(hp, s, p_prev):
                cp, rp = hp // 2, hp % 2
                a2 = avp.tile([128, 512], dt.float32, name="a2", tag="a2")
                for j in range(nkv):
                    nc.tensor.matmul(
                        a2[:],
                        va_sb[j][:, hp * 2 * D:(hp + 1) * 2 * D],
                        p_prev[j][:, ts(s, 512)],
                        start=(j == 0), stop=(j == nkv - 1),
                    )
                # Z replicas (PSUM rows 64-127) -> base-0 SBUF via standard
                # cross-partition copy (custom-DVE recip needs base-0
                # operands); then one DVE mul reads A straight from PSUM.
                zt = npool.tile([64, 512], dt.float32, name="zt", tag="zt")
                nc.vector.tensor_copy(zt[:], a2[D:2 * D, :])
                rz = npool.tile([64, 512], dt.float32, name="rz", tag="rz")
                nc.vector.reciprocal_approx_fast(rz[:], zt[:])
                nc.vector.tensor_mul(
                    aall_sb[cp][rp * 64:(rp + 1) * 64, ts(s, 512)],
                    a2[0:D, :], rz[:],
                )

            with tc.tile_pool(name="ppool", bufs=1) as ppool:
                p_prev = None
                for h in range(HPC + 1):
                    if h < HPC:
                        c, r = h // 2, h % 2
                        qh_h = qh_sb[c][r * 64:(r + 1) * 64, :]
                        kh_h = kh_sb[c][r * 64:(r + 1) * 64, :]
                        p_cur = []
                    target = bg_deadline.get(h, nitems)

                    for half in range(2):
                        if h > 0:
                            # AV strips for the previous head covering this
                            # q-half; frees the P columns exp below rewrites.
                            av_strip(h - 1, 2 * half, p_prev)
                            av_strip(h - 1, 2 * half + 1, p_prev)
                        if h < HPC:
                            for j in range(nkv):
                                for _ in range(2):
                                    nc.tensor.ldweights(weights=wq_sb[0][:, 0:128])
                                if half == 0:
                                    pt = ppool.tile([128, SQ], dt.bfloat16, name=f"p{j}", tag=f"p{j}")
                                    p_cur.append(pt)
                                sc = scp.tile([128, 1024], dt.float32, name="sc", tag="sc")
                                for s in range(2):
                                    nc.tensor.matmul(
                                        sc[:, ts(s, 512)],
                                        kh_h[:, ts(j, 128)],
                                        qh_h[:, half * 1024 + s * 512:half * 1024 + (s + 1) * 512],
                                        start=True, stop=True,
                                    )
                                nc.scalar.activation(
                                    p_cur[j][:, half * 1024:(half + 1) * 1024], sc[:],
                                    mybir.ActivationFunctionType.Exp,
                                    bias=mb_sb[:, j:j + 1], scale=SCALE,
                                )
                                if h == 0 and j % 2 == half:
                                    v_item(j)
                                # pace the background projections
                                want = ((2 * nkv) * target) // (2 * nkv)  # simple full-slot target
                                want = ((half * nkv + j + 1) * target + 2 * nkv - 1) // (2 * nkv)
                                while bg_done < want and bg:
                                    fn, a, b_ = bg.pop()
                                    fn(a, b_)
                                    bg_done += 1

                    p_prev = p_cur if h < HPC else None

            # ---------------- out projection ----------------
            with (
                tc.tile_pool(name="wop", bufs=1) as wop,
                tc.tile_pool(name="opool", bufs=4) as opool,
            ):
                wo_sb = [wop.tile([128, E], dt.bfloat16, name=f"wo{c}", tag=f"wo{c}") for c in range(NDH)]
                for cdh in range(NDH):
                    nc.sync.dma_start(wo_sb[cdh][:], woT[ts(cdh, 128), :])
                for eo in range(NE):
                    for t in range(NTS):
                        ops = pjp.tile([128, 512], dt.float32, name="pj", tag="pj")
                        for cdh in range(NDH):
                            nc.tensor.matmul(
                                ops[:], wo_sb[cdh][:, ts(eo, 128)], aall_sb[cdh][:, ts(t, 512)],
                                start=(cdh == 0), stop=(cdh == NDH - 1),
                            )
                        ob = opool.tile([128, 512], dt.float32, name="ob", tag="ob")
                        nc.vector.tensor_copy(ob[:], ops[:])
                        nc.sync.dma_start(outT[ts(eo, 128), ts(t, 512)], ob[:])

            npool.release()
            pjp.release()
            avp.release()
            scp.release()
            vip.release()

    nc.compile()
    _PROGRAM_CACHE[skv] = nc
    return nc


def make_in_maps(q, k, v, mask, Wq, Wk, Wv, Wo, skv):
    """Host-side shard/compact/transpose/cast. Returns per-core input dicts."""
    in_maps = []
    valid = mask != 0
    for core in range(N_CORES):
        b, hg = core // 2, core % 2
        idx = np.nonzero(valid[b])[0]
        cnt = len(idx)

        kc = np.zeros((skv, E), np.float32)
        vc = np.zeros((skv, E), np.float32)
        kc[:cnt] = k[b][idx]
        vc[:cnt] = v[b][idx]

        mbias = np.zeros((skv,), np.float32)
        mbias[cnt:] = NEG
        # [128, nkv]: column j = kv chunk j
        mb2 = np.ascontiguousarray(mbias.reshape(-1, 128).T)

        rows = slice(hg * DHC, (hg + 1) * DHC)
        in_maps.append(dict(
            qT=np.ascontiguousarray(q[b].T).astype(BF16),
            kT=np.ascontiguousarray(kc.T).astype(BF16),
            vT=np.ascontiguousarray(vc.T).astype(BF16),
            wqT=np.ascontiguousarray(Wq[rows, :].T).astype(BF16),
            wkT=np.ascontiguousarray(Wk[rows, :].T).astype(BF16),
            wvT=np.ascontiguousarray(Wv[rows, :].T).astype(BF16),
            woT=np.ascontiguousarray(Wo[:, rows].T).astype(BF16),
            mb=mb2,
        ))
    return in_maps


def _numpy_fallback(q, k, v, mask, Wq, bq, Wk, bk, Wv, bv, Wo, bo):
    out = np.zeros((B, SQ, E), np.float32)
    for b in range(B):
        qh = (q[b] @ Wq.T + bq).reshape(SQ, H_TOT, D).transpose(1, 0, 2)
        kh = (k[b] @ Wk.T + bk).reshape(-1, H_TOT, D).transpose(1, 0, 2)
        vh = (v[b] @ Wv.T + bv).reshape(-1, H_TOT, D).transpose(1, 0, 2)
        att = np.einsum("hqd,hkd->hqk", qh, kh) * SCALE
        valid = mask[b] != 0
        if not valid.any():
            out[b] = bo
            continue
        att = np.where(valid[None, None, :], att, -np.inf)
        att = att - att.max(-1, keepdims=True)
        att = np.exp(att)
        att /= att.sum(-1, keepdims=True)
        o = np.einsum("hqk,hkd->hqd", att, vh)
        o = o.transpose(1, 0, 2).reshape(SQ, E)
        out[b] = o @ Wo.T + bo
    return out


def kernel(**inputs):
    global LAST_RESULTS
    q = np.asarray(inputs["q"], np.float32)
    k = np.asarray(inputs["k"], np.float32)
    v = np.asarray(inputs["v"], np.float32)
    mask = np.asarray(inputs["mask"])
    Wq, bq = np.asarray(inputs["Wq"], np.float32), np.asarray(inputs["bq"], np.float32)
    Wk, bk = np.asarray(inputs["Wk"], np.float32), np.asarray(inputs["bk"], np.float32)
    Wv, bv = np.asarray(inputs["Wv"], np.float32), np.asarray(inputs["bv"], np.float32)
    Wo, bo = np.asarray(inputs["Wo"], np.float32), np.asarray(inputs["bo"], np.float32)

    if any(np.abs(x).max() > 0 for x in (bq, bk, bv)):
        # q/k/v biases are zero in this problem's setup; a nonzero bias
        # would need the augmented-contraction path, so fall back.
        return _numpy_fallback(q, k, v, mask, Wq, bq, Wk, bk, Wv, bv, Wo, bo)

    valid = mask != 0
    counts = valid.sum(axis=1)
    if counts.max() == 0:
        return np.broadcast_to(bo, (B, SQ, E)).astype(np.float32).copy()

    skv = int(-(-counts.max() // 128) * 128)
    nc = build_program(skv)
    in_maps = make_in_maps(q, k, v, mask, Wq, Wk, Wv, Wo, skv)

    res = bass_utils.run_bass_kernel_spmd(nc, in_maps, core_ids=list(range(N_CORES)))
    LAST_RESULTS = res

    out = np.empty((B, SQ, E), np.float32)
    for b in range(B):
        if counts[b] == 0:
            out[b] = bo
        else:
            p0 = res.results[2 * b]["outT"]
            p1 = res.results[2 * b + 1]["outT"]
            out[b] = p0.T + p1.T + bo
    return out


# revision 13
# speedup vs baseline: 1.7017x; 1.0113x over previous
"""Multi-head attention (batched, key-padding mask) Trainium2 Bass kernel.

Problem: nn_MultiHeadBatched
  q,k,v: [B=4, S=2048, E=1024] fp32; mask: [B, 2048] int32 (key padding)
  16 heads, head_dim 64; torch-Linear style q/k/v/out projections.

Sharding (8 cores): core c handles batch b=c//2 and head group hg=c%2
(8 heads each).  q/k/v projections are column-parallel over the head
group; out-projection is row-parallel — each core produces a partial
[E, Sq] output and the host sums the two partials per batch (+ bo).

v3 structure (single NeuronCore program, SPMD over 8 cores):
  - Host compacts KV per batch to the valid (mask!=0) positions, padded
    to a multiple of 128 (SKV); pad positions get an additive -1e30 exp
    bias (folded into the ScalarE activation).
  - Scores transposed ([kv, q]); softmax denominator Z from an all-ones
    65th column on each head's V (row 64 of the AV accumulation).
  - Head-slot pipeline with HALF-phases: slot h runs
      AV(h-1, strips 0-1) ; scores+exp(h, q-half 0) ;
      AV(h-1, strips 2-3) ; scores+exp(h, q-half 1)
    so the single-buffered P tiles ([128,2048] per kv chunk) free in
    halves just before exp needs them, and ScalarE stays busy across the
    slot boundary (previous half's exps overlap this slot's AV).
  - Q/K projections for head-pairs 1-3 are background items woven into
    the j-loops, so the exp stream starts ~15us into the kernel.
  - AV is strip-major ([65,512] PSUM, 2 banks) and each strip is
    normalized immediately: PSUM->SBUF copy, reciprocal_approx_fast of
    the Z row, GpSimd partition-broadcast + multiply into aall (bf16).
  - PSUM: scores 2x[128,1024] (4) + AV 2x[65,512] (2) + proj 2x[128,512]
    (2) = 8 banks.
"""

import os
import sys

import numpy as np

sys.path.insert(0, "/opt/trn_rl_repo")

import concourse.bass as bass
import concourse.bacc as bacc
import concourse.mybir as mybir
import concourse.tile as tile
from concourse import bass_utils

import ml_dtypes

BF16 = ml_dtypes.bfloat16

B, SQ, E = 4, 2048, 1024
H_TOT, D = 16, 64
HPC = H_TOT // 2            # heads per core (head-group split in 2)
DHC = HPC * D               # 512 projected channels per core
NE = E // 128               # contraction chunks
NDH = DHC // 128            # dh chunks per core
NTS = SQ // 512             # 512-wide q strips
NEG = -1.0e30
SCALE = D ** -0.5

N_CORES = 8

_PROGRAM_CACHE = {}
LAST_RESULTS = None


def _chunks512(n):
    out = []
    o = 0
    while o < n:
        w = min(512, n - o)
        out.append((o, w))
        o += w
    return out


def build_program(skv):
    """Build + compile the single-core SPMD Bass program for padded KV
    length `skv` (multiple of 128)."""
    if skv in _PROGRAM_CACHE:
        return _PROGRAM_CACHE[skv]

    nkv = skv // 128
    dt = mybir.dt

    nc = bacc.Bacc(
        "TRN2",
        target_bir_lowering=False,
        debug=False,
        enable_asserts=False,
        num_devices=N_CORES,
    )

    # DRAM I/O (per-core shapes)
    qT = nc.dram_tensor("qT", [E, SQ], dt.bfloat16, kind="ExternalInput").ap()
    kT = nc.dram_tensor("kT", [E, skv], dt.bfloat16, kind="ExternalInput").ap()
    vT = nc.dram_tensor("vT", [E, skv], dt.bfloat16, kind="ExternalInput").ap()
    wqT = nc.dram_tensor("wqT", [E, DHC], dt.bfloat16, kind="ExternalInput").ap()
    wkT = nc.dram_tensor("wkT", [E, DHC], dt.bfloat16, kind="ExternalInput").ap()
    wvT = nc.dram_tensor("wvT", [E, DHC], dt.bfloat16, kind="ExternalInput").ap()
    woT = nc.dram_tensor("woT", [DHC, E], dt.bfloat16, kind="ExternalInput").ap()
    mb = nc.dram_tensor("mb", [128, nkv], dt.float32, kind="ExternalInput").ap()
    outT = nc.dram_tensor("outT", [E, SQ], dt.float32, kind="ExternalOutput").ap()

    ts = bass.ts
    kvchunks = _chunks512(skv)

    with tile.TileContext(nc) as tc:
        with tc.tile_pool(name="persist", bufs=1) as pp:
            # Persistent SBUF tensors
            wq_sb = [pp.tile([128, DHC], dt.bfloat16, name=f"wq{e}", tag=f"wq{e}") for e in range(NE)]
            wk_sb = [pp.tile([128, DHC], dt.bfloat16, name=f"wk{e}", tag=f"wk{e}") for e in range(NE)]
            wv_sb = [pp.tile([128, DHC], dt.bfloat16, name=f"wv{e}", tag=f"wv{e}") for e in range(NE)]
            qh_sb = [pp.tile([128, SQ], dt.bfloat16, name=f"qh{c}", tag=f"qh{c}") for c in range(NDH)]
            kh_sb = [pp.tile([128, skv], dt.bfloat16, name=f"kh{c}", tag=f"kh{c}") for c in range(NDH)]
            # V augmented per head to [kv, 64 V | 64 ones]: the ones block
            # replicates the softmax denominator Z into PSUM rows 64-127.
            va_sb = [pp.tile([128, HPC * 2 * D], dt.bfloat16, name=f"va{j}", tag=f"va{j}") for j in range(nkv)]
            aall_sb = [pp.tile([128, SQ], dt.bfloat16, name=f"aall{c}", tag=f"aall{c}") for c in range(NDH)]
            mb_sb = pp.tile([128, nkv], dt.float32, name="mbt", tag="mbt")

            # ones half-blocks of the augmented V (bf16 1.0)
            for j in range(nkv):
                va3 = va_sb[j].rearrange("p (h x) -> p h x", x=2 * D)
                nc.gpsimd.memset(va3[:, :, D:2 * D], 1.0)

            vip = tc.alloc_tile_pool(name="vinp", bufs=1)
            q_sb = [vip.tile([128, SQ], dt.bfloat16, name=f"q{e}", tag=f"q{e}") for e in range(NE)]
            k_sb = [vip.tile([128, skv], dt.bfloat16, name=f"k{e}", tag=f"k{e}") for e in range(NE)]
            v_sb = [vip.tile([128, skv], dt.bfloat16, name=f"v{e}", tag=f"v{e}") for e in range(NE)]

            # DMA order matches first-use order
            for e in range(NE):
                nc.sync.dma_start(wq_sb[e][:], wqT[ts(e, 128), :])
                nc.sync.dma_start(q_sb[e][:], qT[ts(e, 128), :])
            for e in range(NE):
                nc.sync.dma_start(wk_sb[e][:], wkT[ts(e, 128), :])
                nc.sync.dma_start(k_sb[e][:], kT[ts(e, 128), :])
            nc.sync.dma_start(mb_sb[:], mb[:])
            for e in range(NE):
                nc.sync.dma_start(wv_sb[e][:], wvT[ts(e, 128), :])
                nc.sync.dma_start(v_sb[e][:], vT[ts(e, 128), :])

            # PSUM pools, alive for the whole program
            scp = tc.alloc_tile_pool(name="scp", bufs=2, space="PSUM")
            avp = tc.alloc_tile_pool(name="avp", bufs=2, space="PSUM")
            pjp = tc.alloc_tile_pool(name="pjp", bufs=2, space="PSUM")
            npool = tc.alloc_tile_pool(name="npool", bufs=2)

            # ---------------- work items ----------------
            def q_item(c, t):
                qps = pjp.tile([128, 512], dt.float32, name="pj", tag="pj")
                for e in range(NE):
                    nc.tensor.matmul(
                        qps[:], wq_sb[e][:, ts(c, 128)], q_sb[e][:, ts(t, 512)],
                        start=(e == 0), stop=(e == NE - 1),
                    )
                nc.vector.tensor_copy(qh_sb[c][:, ts(t, 512)], qps[:])

            def k_item(c, ci):
                o, w = kvchunks[ci]
                kps = pjp.tile([128, 512], dt.float32, name="pj", tag="pj")
                for e in range(NE):
                    nc.tensor.matmul(
                        kps[:, 0:w], wk_sb[e][:, ts(c, 128)], k_sb[e][:, o:o + w],
                        start=(e == 0), stop=(e == NE - 1),
                    )
                nc.vector.tensor_copy(kh_sb[c][:, o:o + w], kps[:, 0:w])

            def v_item(j):
                vps = pjp.tile([128, 512], dt.float32, name="pj", tag="pj")
                for e in range(NE):
                    nc.tensor.matmul(
                        vps[:], v_sb[e][:, ts(j, 128)], wv_sb[e][:],
                        start=(e == 0), stop=(e == NE - 1),
                    )
                dst = va_sb[j].rearrange("p (h x) -> p h x", x=2 * D)[:, :, 0:D]
                src = vps.rearrange("p (h x) -> p h x", x=D)
                nc.vector.tensor_copy(dst, src)

            # Background queue: Q/K projections for pairs 1-3, popped inside
            # the slot j-loops.  Pair c is fully drained well before slot 2c.
            bg = []
            for c in range(1, NDH):
                for t in range(NTS):
                    bg.append((q_item, c, t))
                for ci in range(len(kvchunks)):
                    bg.append((k_item, c, ci))
            bg.reverse()  # pop() from the end
            nitems = len(bg)
            # cumulative items to drain by end of slot h (pair c by slot 2c-1)
            per_pair = nitems // 3
            bg_deadline = {0: 4, 1: per_pair, 2: per_pair + 4, 3: 2 * per_pair,
                           4: 2 * per_pair + 4, 5: nitems}
            bg_done = 0

            # ---------------- prologue: pair-0 projections ----------------
            for t in range(NTS):
                q_item(0, t)
            for ci in range(len(kvchunks)):
                k_item(0, ci)

            # ---------------- head-slot pipeline ----------------
            def av_half(hp, half, p_prev):
                # AV for q-half `half` of head hp, j-major over a single
                # [128,1024] PSUM tile (one weight load per kv chunk).
                cp, rp = hp // 2, hp % 2
                a2 = avp.tile([128, 1024], dt.float32, name="a2", tag="a2", bufs=1)
                for j in range(nkv):
                    for s in range(2):
                        nc.tensor.matmul(
                            a2[:, ts(s, 512)],
                            va_sb[j][:, hp * 2 * D:(hp + 1) * 2 * D],
                            p_prev[j][:, half * 1024 + s * 512:half * 1024 + (s + 1) * 512],
                            start=(j == 0), stop=(j == nkv - 1),
                        )
                # Z replicas (PSUM rows 64-127) -> base-0 SBUF via standard
                # cross-partition copy (custom-DVE recip needs base-0
                # operands); then one DVE mul reads A straight from PSUM.
                zt = npool.tile([64, 1024], dt.float32, name="zt", tag="zt")
                nc.vector.tensor_copy(zt[:], a2[D:2 * D, :])
                rz = npool.tile([64, 1024], dt.float32, name="rz", tag="rz")
                nc.vector.reciprocal_approx_fast(rz[:], zt[:])
                nc.vector.tensor_mul(
                    aall_sb[cp][rp * 64:(rp + 1) * 64, half * 1024:(half + 1) * 1024],
                    a2[0:D, :], rz[:],
                )

            with tc.tile_pool(name="ppool", bufs=1) as ppool:
                p_prev = None
                for h in range(HPC + 1):
                    if h < HPC:
                        c, r = h // 2, h % 2
                        qh_h = qh_sb[c][r * 64:(r + 1) * 64, :]
                        kh_h = kh_sb[c][r * 64:(r + 1) * 64, :]
                        p_cur = []
                    target = bg_deadline.get(h, nitems)

                    for half in range(2):
                        if h > 0:
                            # AV for the previous head covering this q-half;
                            # frees the P columns exp below rewrites.
                            av_half(h - 1, half, p_prev)
                        if h < HPC:
                            for j in range(nkv):
                                if h >= 5:
                                    nc.tensor.ldweights(weights=wq_sb[0][:, 0:128])
                                if half == 0:
                                    pt = ppool.tile([128, SQ], dt.bfloat16, name=f"p{j}", tag=f"p{j}")
                                    p_cur.append(pt)
                                sc = scp.tile([128, 1024], dt.float32, name="sc", tag="sc")
                                for s in range(2):
                                    nc.tensor.matmul(
                                        sc[:, ts(s, 512)],
                                        kh_h[:, ts(j, 128)],
                                        qh_h[:, half * 1024 + s * 512:half * 1024 + (s + 1) * 512],
                                        start=True, stop=True,
                                    )
                                nc.scalar.activation(
                                    p_cur[j][:, half * 1024:(half + 1) * 1024], sc[:],
                                    mybir.ActivationFunctionType.Exp,
                                    bias=mb_sb[:, j:j + 1], scale=SCALE,
                                )
                                if h == 0 and j % 2 == half:
                                    v_item(j)
                                # pace the background projections
                                want = ((2 * nkv) * target) // (2 * nkv)  # simple full-slot target
                                want = ((half * nkv + j + 1) * target + 2 * nkv - 1) // (2 * nkv)
                                while bg_done < want and bg:
                                    fn, a, b_ = bg.pop()
                                    fn(a, b_)
                                    bg_done += 1

                    p_prev = p_cur if h < HPC else None

            # ---------------- out projection ----------------
            with (
                tc.tile_pool(name="wop", bufs=1) as wop,
                tc.tile_pool(name="opool", bufs=4) as opool,
            ):
                wo_sb = [wop.tile([128, E], dt.bfloat16, name=f"wo{c}", tag=f"wo{c}") for c in range(NDH)]
                for cdh in range(NDH):
                    nc.sync.dma_start(wo_sb[cdh][:], woT[ts(cdh, 128), :])
                for eo in range(NE):
                    for t in range(NTS):
                        ops = pjp.tile([128, 512], dt.float32, name="pj", tag="pj")
                        for cdh in range(NDH):
                            nc.tensor.matmul(
                                ops[:], wo_sb[cdh][:, ts(eo, 128)], aall_sb[cdh][:, ts(t, 512)],
                                start=(cdh == 0), stop=(cdh == NDH - 1),
                            )
                        ob = opool.tile([128, 512], dt.float32, name="ob", tag="ob")
                        nc.vector.tensor_copy(ob[:], ops[:])
                        nc.sync.dma_start(outT[ts(eo, 128), ts(t, 512)], ob[:])

            npool.release()
            pjp.release()
            avp.release()
            scp.release()
            vip.release()

    nc.compile()
    _PROGRAM_CACHE[skv] = nc
    return nc


def make_in_maps(q, k, v, mask, Wq, Wk, Wv, Wo, skv):
    """Host-side shard/compact/transpose/cast. Returns per-core input dicts."""
    in_maps = []
    valid = mask != 0
    for core in range(N_CORES):
        b, hg = core // 2, core % 2
        idx = np.nonzero(valid[b])[0]
        cnt = len(idx)

        kc = np.zeros((skv, E), np.float32)
        vc = np.zeros((skv, E), np.float32)
        kc[:cnt] = k[b][idx]
        vc[:cnt] = v[b][idx]

        mbias = np.zeros((skv,), np.float32)
        mbias[cnt:] = NEG
        # [128, nkv]: column j = kv chunk j
        mb2 = np.ascontiguousarray(mbias.reshape(-1, 128).T)

        rows = slice(hg * DHC, (hg + 1) * DHC)
        in_maps.append(dict(
            qT=np.ascontiguousarray(q[b].T).astype(BF16),
            kT=np.ascontiguousarray(kc.T).astype(BF16),
            vT=np.ascontiguousarray(vc.T).astype(BF16),
            wqT=np.ascontiguousarray(Wq[rows, :].T).astype(BF16),
            wkT=np.ascontiguousarray(Wk[rows, :].T).astype(BF16),
            wvT=np.ascontiguousarray(Wv[rows, :].T).astype(BF16),
            woT=np.ascontiguousarray(Wo[:, rows].T).astype(BF16),
            mb=mb2,
        ))
    return in_maps


def _numpy_fallback(q, k, v, mask, Wq, bq, Wk, bk, Wv, bv, Wo, bo):
    out = np.zeros((B, SQ, E), np.float32)
    for b in range(B):
        qh = (q[b] @ Wq.T + bq).reshape(SQ, H_TOT, D).transpose(1, 0, 2)
        kh = (k[b] @ Wk.T + bk).reshape(-1, H_TOT, D).transpose(1, 0, 2)
        vh = (v[b] @ Wv.T + bv).reshape(-1, H_TOT, D).transpose(1, 0, 2)
        att = np.einsum("hqd,hkd->hqk", qh, kh) * SCALE
        valid = mask[b] != 0
        if not valid.any():
            out[b] = bo
            continue
        att = np.where(valid[None, None, :], att, -np.inf)
        att = att - att.max(-1, keepdims=True)
        att = np.exp(att)
        att /= att.sum(-1, keepdims=True)
        o = np.einsum("hqk,hkd->hqd", att, vh)
        o = o.transpose(1, 0, 2).reshape(SQ, E)
        out[b] = o @ Wo.T + bo
    return out


def kernel(**inputs):
    global LAST_RESULTS
    q = np.asarray(inputs["q"], np.float32)
    k = np.asarray(inputs["k"], np.float32)
    v = np.asarray(inputs["v"], np.float32)
    mask = np.asarray(inputs["mask"])
    Wq, bq = np.asarray(inputs["Wq"], np.float32), np.asarray(inputs["bq"], np.float32)
    Wk, bk = np.asarray(inputs["Wk"], np.float32), np.asarray(inputs["bk"], np.float32)
    Wv, bv = np.asarray(inputs["Wv"], np.float32), np.asarray(inputs["bv"], np.float32)
    Wo, bo = np.asarray(inputs["Wo"], np.float32), np.asarray(inputs["bo"], np.float32)

    if any(np.abs(x).max() > 0 for x in (bq, bk, bv)):
        # q/k/v biases are zero in this problem's setup; a nonzero bias
        # would need the augmented-contraction path, so fall back.
        return _numpy_fallback(q, k, v, mask, Wq, bq, Wk, bk, Wv, bv, Wo, bo)

    valid = mask != 0
    counts = valid.sum(axis=1)
    if counts.max() == 0:
        return np.broadcast_to(bo, (B, SQ, E)).astype(np.float32).copy()

    skv = int(-(-counts.max() // 128) * 128)
    nc = build_program(skv)
    in_maps = make_in_maps(q, k, v, mask, Wq, Wk, Wv, Wo, skv)

    res = bass_utils.run_bass_kernel_spmd(nc, in_maps, core_ids=list(range(N_CORES)))
    LAST_RESULTS = res

    out = np.empty((B, SQ, E), np.float32)
    for b in range(B):
        if counts[b] == 0:
            out[b] = bo
        else:
            p0 = res.results[2 * b]["outT"]
            p1 = res.results[2 * b + 1]["outT"]
            out[b] = p0.T + p1.T + bo
    return out
